# revision 1
# baseline (speedup 1.0000x reference)
"""Biaffine scorer kernel for Trainium2 (Bass/Tile), data-parallel over batch
across 8 NeuronCores — bf16 pipeline, c-group software pipelining (v3).

See kernel_v2.py for the bf16 rationale. v3 restructures the per-pair schedule
to keep every engine streaming:

  proj_t (8 mm) -> proj_h (8 mm)            # t first: tUT only needs t1T
  for c in 0..5:                            # software pipeline, 1-group lag
      c<5:  tut mm o=2c, 2c+1 -> ACT copies into tut[:, (o, b01, y)]
      c>=1: finals group c-1: for each of 4 (bb, xt) tiles,
            2 N=256 matmuls into one PSUM bank, then evac+cmat-add routed
            3x DVE-direct / 1x ACT-copy+GpSimd-add (rotating)
      output tiles DMA'd in 2 column-halves (after chunks 2 and 4)

This removes the ACT-bound tUT burst / DVE-bound finals burst alternation of
v2 — the PE never waits on a just-issued evacuation, so it stays continuously
busy and ramps to its full 2.4 GHz p-state.
"""

import numpy as np
import ml_dtypes

import concourse.bass as bass
import concourse.bacc as bacc
import concourse.tile as tile
from concourse import mybir
from concourse.bass_utils import run_bass_kernel_spmd

# problem shape (hardcoded per harness contract)
B, S, H = 32, 255, 1024
BS, WD, O = 120, 20, 10
SP = 256            # padded S
SP2 = 2 * SP        # paired moving dim
NW = SP * O         # 2560
NREAL = S * O       # 2550
KT = H // 128       # 8
NCORES = 8
BPC = B // NCORES   # 4 batch items per core
NP = BPC // 2       # 2 pairs per core
BSE = BS + 1        # 121
UTW = O * BSE + 6   # 1216 (pad to keep 4B-aligned rows)

F32 = mybir.dt.float32
F16 = mybir.dt.bfloat16
BF16NP = ml_dtypes.bfloat16

_CACHE: dict = {}


def _emit(tc, d):
    """Emit the per-core program. d: dict of DRAM APs."""
    from contextlib import ExitStack

    nc = tc.nc
    AF = mybir.ActivationFunctionType

    with ExitStack() as ctx:
        const = ctx.enter_context(tc.tile_pool(name="const", bufs=1))
        st_pool = ctx.enter_context(tc.tile_pool(name="st", bufs=NP * 4))
        ht_pool = ctx.enter_context(tc.tile_pool(name="ht", bufs=4))
        tut_pool = ctx.enter_context(tc.tile_pool(name="tut", bufs=2))
        out_pool = ctx.enter_context(tc.tile_pool(name="outp", bufs=8))
        pp_ht = ctx.enter_context(tc.tile_pool(name="pp_ht", bufs=2, space="PSUM"))
        pp_u = ctx.enter_context(tc.tile_pool(name="pp_u", bufs=2, space="PSUM"))
        pp_s = ctx.enter_context(tc.tile_pool(name="pp_s", bufs=4, space="PSUM"))

        # ---- persistent constants + stateT prefetch, all on the SP HWDGE
        # ring, in consumption-priority order (hw + first quarter feed the
        # first matmul; cmat is first read ~7us in; outputs queue behind) ----
        sb_hw = const.tile([128, KT * BSE], F16)
        sb_tw = const.tile([128, KT * BSE], F16)
        sb_bias = const.tile([BSE, 2], F32)
        # ut: per-o [121, 121] blocks (Wt in col 120, Wh folded into row 120)
        sb_ut = const.tile([BSE, UTW], F16)
        sb_c0 = const.tile([128, NW], F16)
        sb_c1 = const.tile([128, NW], F16)
        stq = [
            st_pool.tile([128, 2 * SP2], F16, name=f"stq_{p}_{q}", tag="stq")
            for p in range(NP)
            for q in range(4)
        ]
        loads = [
            (sb_hw[:], d["hw"]),
            (stq[0][:], d["stateT"][0][:, 0:1024]),
            (sb_tw[:], d["tw"]),
            (stq[1][:], d["stateT"][0][:, 1024:2048]),
            (sb_bias[:], d["bias2"]),
            (sb_ut[:], d["ut"]),
            (stq[2][:], d["stateT"][0][:, 2048:3072]),
            (stq[3][:], d["stateT"][0][:, 3072:4096]),
            (sb_c0[:], d["cmat"][0:128, :]),
            (sb_c1[:], d["cmat"][128:256, :]),
            (stq[4][:], d["stateT"][1][:, 0:1024]),
            (stq[5][:], d["stateT"][1][:, 1024:2048]),
            (stq[6][:], d["stateT"][1][:, 2048:3072]),
            (stq[7][:], d["stateT"][1][:, 3072:4096]),
        ]
        for dst, src_ in loads:
            nc.sync.dma_start(dst, src_)

        # ---- PE warm-up: the tensor engine needs ~3us of continuous busy
        # to leave its 1.2GHz mid p-state. These dummies depend only on a
        # DVE memset, so they run during the input-DMA head; real proj
        # matmuls then start already at 2.4GHz (426ns -> 213ns per MM).
        scratch = const.tile([128, 512], F16)
        nc.vector.memset(scratch[:], 0)
        for i in range(12):
            ps_d = pp_ht.tile([128, 512], F32, name=f"dmy_{i}", tag="ps")
            nc.tensor.matmul(
                ps_d[:], lhsT=scratch[:, 0:128], rhs=scratch[:],
                start=True, stop=True,
            )

        kevac = 0  # global finals-chunk counter -> rotates the AP route slot
        for p in range(NP):
            # ---- projections: t first (tUT depends only on t1T) ----
            h1T = None
            t1T = None
            for which, w, bcol in ((1, sb_tw, 1), (0, sb_hw, 0)):
                ps = pp_ht.tile([BSE, SP2], F32, name=f"ps_p{p}_{which}", tag="ps")
                for kt in range(KT):
                    st = stq[p * 4 + kt // 2]
                    nc.tensor.matmul(
                        ps[:],
                        lhsT=w[:, kt * BSE:(kt + 1) * BSE],
                        rhs=st[:, (kt % 2) * SP2:(kt % 2 + 1) * SP2],
                        start=(kt == 0),
                        stop=(kt == KT - 1),
                    )
                hv = ht_pool.tile([BSE, SP2], F16, name=f"ht_p{p}_{which}", tag="hv")
                # leaky(psum + bias); row 120: weights col is 0, bias 1 -> 1.0
                nc.scalar.activation(
                    hv[:], ps[:], AF.Lrelu,
                    bias=sb_bias[:, bcol:bcol + 1], scale=1.0, alpha=0.01,
                )
                if which:
                    t1T = hv
                else:
                    h1T = hv

            # ---- software-pipelined c-groups ----
            tut = tut_pool.tile([BSE, O * SP2], F16)
            outs = [
                out_pool.tile([128, NW], F16, name=f"sb_out_p{p}_{i}", tag="sb_out")
                for i in range(4)
            ]
            for c in range(6):
                if c < 5:
                    # produce tut columns for o = 2c, 2c+1
                    for half in range(2):
                        o = 2 * c + half
                        ps_u = pp_u.tile([BSE, SP2], F32, name=f"ps_u_{p}_{o}", tag="ps_u")
                        nc.tensor.matmul(
                            ps_u[:],
                            lhsT=sb_ut[:, o * BSE:(o + 1) * BSE],
                            rhs=t1T[:],
                            start=True,
                            stop=True,
                        )
                        tdst = tut[:, o * SP2:(o + 1) * SP2]
                        if c == 2:
                            nc.vector.tensor_scalar_add(tdst, ps_u[:], 0.0)
                        else:
                            nc.scalar.activation(tdst, ps_u[:], AF.Copy)
                if c >= 1:
                    cc = c - 1  # finals for the previous tut group
                    for i in range(4):
                        bb, xt = i // 2, i % 2
                        sb_c = sb_c0 if xt == 0 else sb_c1
                        sb_out = outs[i]
                        lo = bb * SP + xt * 128
                        ps_s = pp_s.tile([128, 512], F32, name=f"ps_s_{p}_{cc}_{i}", tag="ps_s")
                        for half in range(2):
                            o = 2 * cc + half
                            nc.tensor.matmul(
                                ps_s[:, half * 256:(half + 1) * 256],
                                lhsT=h1T[:, lo:lo + 128],
                                rhs=tut[:, o * SP2 + bb * SP:o * SP2 + bb * SP + SP],
                                start=True,
                                stop=True,
                            )
                        oc = sb_out[:, cc * 512:(cc + 1) * 512]
                        co = sb_c[:, cc * 512:(cc + 1) * 512]
                        if kevac % 4 == 3 and kevac < 36:
                            # every 4th chunk: ACT copy + GpSimd in-place add
                            nc.scalar.activation(oc, ps_s[:], AF.Copy)
                            nc.gpsimd.tensor_add(oc, oc, co)
                        else:
                            nc.vector.tensor_add(oc, ps_s[:], co)
                        kevac += 1
                        # stream the output in 3 column pieces per tile;
                        # the small last piece shortens the drain tail
                        pieces = {1: (0, 1024), 3: (1024, 2048), 4: (2048, NW)}
                        if cc in pieces:
                            a, bnd = pieces[cc]
                            nc.sync.dma_start(
                                d["out"][2 * p + bb, xt * 128:(xt + 1) * 128, a:bnd],
                                sb_out[:, a:bnd],
                            )


def build_nc():
    if "nc" in _CACHE:
        return _CACHE["nc"]
    nc = bacc.Bacc(
        "TRN2", target_bir_lowering=False, debug=False, num_devices=NCORES
    )
    d = {}
    d["stateT"] = nc.dram_tensor(
        "stateT", [NP, 128, KT * SP2], F16, kind="ExternalInput"
    ).ap()
    d["hw"] = nc.dram_tensor("hw", [128, KT * BSE], F16, kind="ExternalInput").ap()
    d["tw"] = nc.dram_tensor("tw", [128, KT * BSE], F16, kind="ExternalInput").ap()
    d["ut"] = nc.dram_tensor("ut", [BSE, UTW], F16, kind="ExternalInput").ap()
    d["bias2"] = nc.dram_tensor("bias2", [BSE, 2], F32, kind="ExternalInput").ap()
    d["cmat"] = nc.dram_tensor("cmat", [SP, NW], F16, kind="ExternalInput").ap()
    d["out"] = nc.dram_tensor("out", [BPC, SP, NW], F16, kind="ExternalOutput").ap()

    with tile.TileContext(nc) as tc:
        _emit(tc, d)
    nc.compile()
    _CACHE["nc"] = nc
    return nc


def prep_inputs(inputs):
    """Host-side packing + fp32->bf16 conversion. Returns dict of np arrays
    shared across cores (stateT is full-batch; shard before dispatch)."""
    state = np.asarray(inputs["state"], np.float32)
    head_w = np.asarray(inputs["head_w"], np.float32)
    head_b = np.asarray(inputs["head_b"], np.float32)
    tail_w = np.asarray(inputs["tail_w"], np.float32)
    tail_b = np.asarray(inputs["tail_b"], np.float32)
    U = np.asarray(inputs["U"], np.float32)
    width_table = np.asarray(inputs["width_table"], np.float32)
    cls_w = np.asarray(inputs["cls_w"], np.float32)
    cls_b = np.asarray(inputs["cls_b"], np.float32)

    # stateT paired pack: [B/2, 128, (kt, b01, y)], y zero-padded to 256
    stateT = np.zeros((B, H, SP), np.float32)
    stateT[:, :, :S] = state.transpose(0, 2, 1)
    stateT = stateT.reshape(B // 2, 2, KT, 128, SP).transpose(0, 3, 2, 1, 4)
    stateT = np.ascontiguousarray(
        stateT.reshape(B // 2, 128, KT * SP2).astype(BF16NP)
    )

    hw_sb = np.zeros((128, KT, BSE), np.float32)
    hw_sb[:, :, :BS] = head_w.reshape(KT, 128, BS).transpose(1, 0, 2)
    hw_sb = np.ascontiguousarray(hw_sb.reshape(128, KT * BSE).astype(BF16NP))
    tw_sb = np.zeros((128, KT, BSE), np.float32)
    tw_sb[:, :, :BS] = tail_w.reshape(KT, 128, BS).transpose(1, 0, 2)
    tw_sb = np.ascontiguousarray(tw_sb.reshape(128, KT * BSE).astype(BF16NP))

    # ut blocks: [j, o, i] = U[o,i,j]; col 120 = Wt_ext; row 120 += Wh_ext
    ut = np.zeros((BSE, UTW), np.float32)
    blocks = ut[:, :O * BSE].reshape(BSE, O, BSE)
    blocks[:BS, :, :BS] = U.transpose(2, 0, 1)
    blocks[:, :, BS] = cls_w[:, BS + 1:2 * (BS + 1)].T
    blocks[BS, :, :] += cls_w[:, :BSE]
    ut = np.ascontiguousarray(ut.astype(BF16NP))

    bias2 = np.zeros((BSE, 2), np.float32)
    bias2[:BS, 0] = head_b
    bias2[BS, 0] = 1.0
    bias2[:BS, 1] = tail_b
    bias2[BS, 1] = 1.0

    # cmat[x, o*256+y] = wproj[pos(x,y), o], wproj = width_table@Ww.T + cls_b
    pos = np.arange(S)[None, :] - np.arange(S)[:, None] + 1
    pos = pos * (pos > 0)
    posP = np.zeros((SP, SP), np.int64)
    posP[:S, :S] = pos
    wproj = width_table @ cls_w[:, 2 * (BS + 1):].T + cls_b   # [256, 10]
    cmat = wproj[posP]                       # [x, y, o]
    cmat = np.ascontiguousarray(
        cmat.transpose(0, 2, 1).reshape(SP, NW).astype(BF16NP)
    )

    return {
        "stateT": stateT,
        "hw": hw_sb,
        "tw": tw_sb,
        "ut": ut,
        "bias2": bias2,
        "cmat": cmat,
    }


def run(inputs, trace=False, trace_kwargs=None):
    nc = build_nc()
    full = prep_inputs(inputs)
    shared = {k: v for k, v in full.items() if k != "stateT"}
    in_maps = []
    for c in range(NCORES):
        m = dict(shared)
        m["stateT"] = np.ascontiguousarray(full["stateT"][c * NP:(c + 1) * NP])
        in_maps.append(m)
    res = run_bass_kernel_spmd(
        nc,
        in_maps,
        core_ids=list(range(NCORES)),
        trace=trace,
        **(trace_kwargs or {}),
    )
    out = np.concatenate([r["out"] for r in res.results], axis=0)
    # [B, x(256), (o,y)] bf16 -> [B, x, y, o] fp32, trim padding
    out = out.astype(np.float32).reshape(B, SP, O, SP)
    out = np.ascontiguousarray(out.transpose(0, 1, 3, 2)[:, :S, :S, :])
    return out, res


def kernel(**inputs):
    out, _ = run(inputs, trace=False)
    return out


if __name__ == "__main__":
    build_nc()
    print("build ok")

